# revision 62
# baseline (speedup 1.0000x reference)
"""Trainium2 Bass kernel for involution-style aggregation (SAN Aggregation).

Per batch element b (one per NeuronCore, pure data parallel over B=8):
    out[c, p] = sum_{idx in 0..8} x[c, p + 64*di + dj] * w[c % 16, idx, p]
with (di, dj) = (idx // 3 - 1, idx % 3 - 1), zero padding 1, K=3, stride 1.

Layout (the key trick): SBUF partition q = wc*8 + rb packs the 16 weight
channels x 8 row-blocks; the free dim per 128-channel block is
m = rr*512 + g*64 + j (rr = row-in-rowblock 0..7, g = channel group 0..7,
j = column 0..63).  In this layout the weight for output (q, rr, g, j) is
wt[q, rr*64 + j] -- a COMPACT [128, 512] tile per tap consumed via a
stride-0 broadcast access pattern ([[64,nrr],[0,8],[1,64]]), so weights are
never replicated (1.2 MB total weight DMA instead of 9 MB, and no PE
broadcast pipeline).  Verified bit-exact on hardware.

- Row shifts di stay inside a partition for rr+di in [0,8) (flat offset
  512*di + dj); the one boundary row per row-block reads from small staged
  tiles xup/xdn ([128, 514] per block) whose rb=0 / rb=7 rows are zero, so
  row-edge padding needs no clipping anywhere.  Column-edge padding is
  folded into the weights host-side (dj=-1 taps zero j==0, dj=+1 taps zero
  j==63), so wrapped reads multiply by zero.  Every tap contributes a full
  densely-written plane.
- The ~73.7K column-multiplies split between DVE (0.52 ns/col, fp16 2x
  mode) and GPSIMD (0.83 ns/col) via an arrival-aware greedy balance
  (~28us each) -- the two-engine multiply roofline for this model.
- Tap accumulation is offloaded from DVE/GPSIMD: regions A=(blk0,lo),
  B=(blk0,hi), C=(blk1,lo) accumulate on the TensorEngine as 512-wide
  identity matmuls into [128,1024] f32 PSUM quarter tiles; region
  D=(blk1,hi) takes len(d_pe) taps on PE (its own PSUM pair, drained
  mid-kernel by ACT) and the rest as pinned f16 quarter-adds on
  DVE/GPSIMD.  PSUM bank schedule (8 banks = 2 region pairs):
  ps_a pair hosts A then C; ps_d pair hosts D's partial then B.
- Emission is a merged walk keeping program order topological while each
  engine's queue follows its own schedule: V/G feed order interleaves D4
  with early block0 taps (PE's in-order conveyor never starves), then
  block0 with C taps and D-adds interspersed; PE consumes warm-up, D4,
  then region-major A (closing A early for C's banks), B from buffered
  tmps, C; ACT runs drains the moment regions close.  The kernel ends on
  C's final (di=0) tap emitted in eighths: multiply -> stop-chunk ->
  drain -> store pipelined so only ~1.5us of drain chain sits in the tail.
- DMA is issued from both SP (sync) and ACT (scalar) queues, which execute
  in parallel in this cost model; DMA occupies the issuing engine, so the
  compact-weight layout is what keeps the queues short.  x loads are
  quarter-chunked and ordered (with one hoisted early weight filling the
  initial DVE stall) so compute starts at ~2.9us on both vector engines.
- A PE warm train (16 tiny matmuls) walks the TensorEngine p-state ramp
  during the initial x-load window so real matmuls run at 2.4 GHz.
- Output is stored fp16 in the permuted layout and un-permuted on the
  host.  ~37.7us/core simulated vs the 57.0us previous baseline;
  scale-relative error ~1.0e-3 (gate 2e-2).
- _legalize_sync_waits rewrites the scheduled IR so no instruction carries
  more than one sync wait (walrus codegen limit in this toolchain).
"""

import sys

for _p in (
    "/root/.axon_site",
    "/root/.axon_site/_ro/trn_rl_repo",
    "/root/.axon_site/_ro/pypackages",
):
    if _p not in sys.path:
        sys.path.append(_p)

from contextlib import ExitStack

import numpy as np

import concourse.bass as bass
import concourse.tile as tile
from concourse import mybir
from concourse.bass_utils import run_bass_kernel_spmd

B, C, H, W = 8, 256, 64, 64
WC, K2 = 16, 9
OH, OW = 64, 64
P = OH * OW
N_CORES = 8
F32 = mybir.dt.float32
F16 = mybir.dt.float16
HALF = P // 2  # 2048
QUAR = P // 4  # 1024

# tap processing order within each phase: di=0 first (no xe dependency),
# then di=+1 (needs xdn), then di=-1 (needs xup)
TAP_ORDER = [4, 3, 5, 7, 6, 8, 1, 0, 2]
# number of region-D taps (beyond the center tap) accumulated on PE before
# the mid-kernel drain; the remaining 8-K_PE taps become DVE/GPSIMD adds
K_PE = 3
# engine cost constants for the greedy DVE/GPSIMD balance
EV, EG = 0.5208, 0.8333
EV_FIX = 60.0


def _tap_geom(idx):
    di, dj = idx // 3 - 1, idx % 3 - 1
    return di, dj


def _legalize_sync_waits(nc, max_waits: int = 1) -> int:
    """Walrus codegen rejects instructions with >1 sync wait. Hoist excess
    waits onto same-engine NoOp carriers inserted just before the
    over-subscribed instruction (per-engine program order preserved)."""
    n_moved = 0
    counter = [0]
    for func in nc.m.functions:
        for bb in func.blocks:
            insts = list(bb.instructions)
            out = []
            changed = False
            for inst in insts:
                si = inst.sync_info
                waits = list(si.on_wait) if (si and si.on_wait) else []
                if len(waits) > max_waits:
                    extra, keep = waits[:-max_waits], waits[-max_waits:]
                    for w in extra:
                        counter[0] += 1
                        carrier = mybir.InstNoOp(
                            name=f"{inst.name}_wsplit{counter[0]}", ins=[], outs=[]
                        )
                        carrier.engine = inst.engine
                        carrier.sync_info = mybir.SyncInfo(on_wait=[w], on_update=[])
                        out.append(carrier)
                        n_moved += 1
                    si.on_wait = keep
                    changed = True
                out.append(inst)
            if changed:
                try:
                    bb.instructions = out
                except Exception:
                    cur = bb.instructions
                    cur[:] = out
    return n_moved


DEFAULT_CFG = dict(
    warm=16,
    drain_est=15000.0,
    drain_inc=1400.0,
    eng_init=2300.0,
    tpf_bufs=8,
    d_pe=(4, 3, 5, 7),
    tap_order=(4, 3, 5, 7, 6, 8, 1, 0, 2),
    c_order=(4, 3, 7, 6, 8, 1, 0, 2, 5),
    v_bias=800.0,
    pin_adds=True,
    add_pos=8,
    c_start=6,
    c_early=4,
    b_lag=9,
)


def _build(legalize: bool = True, cfg: dict | None = None):
    cfg = {**DEFAULT_CFG, **(cfg or {})}
    nc = bass.Bass()
    xb_d = nc.declare_dram_parameter("xb", [2, 128, P + 2], F16, isOutput=False)
    xe_d = nc.declare_dram_parameter("xe", [2, 2, 128, 514], F16, isOutput=False)
    wt_d = nc.declare_dram_parameter("wt", [K2, 128, 512], F16, isOutput=False)
    out = nc.declare_dram_parameter("out", [2, 128, P], F16, isOutput=True)
    ident_d = nc.inline_tensor(np.eye(128, dtype=np.float16), name="ident")

    # per-engine projected clocks for the assignment heuristic; ready_est
    # models input-DMA arrival for the first ops so early units spread
    # across both engines instead of piling onto whichever is "emptier".
    eng_busy = {"v": cfg["eng_init"], "g": cfg["eng_init"]}
    ready_hint = [0.0]

    def pick_engine(nels):
        r = ready_hint[0]
        ev = max(eng_busy["v"], r) + nels * EV + EV_FIX
        eg = max(eng_busy["g"], r) + nels * EG
        if ev <= eg:
            eng_busy["v"] = ev
            return nc.vector
        eng_busy["g"] = eg
        return nc.gpsimd

    with tile.TileContext(nc) as tc:
        with ExitStack() as ctx:
            idp = ctx.enter_context(tc.tile_pool(name="idp", bufs=1))
            xp = ctx.enter_context(tc.tile_pool(name="xp", bufs=1))
            wp = ctx.enter_context(tc.tile_pool(name="wp", bufs=1))
            tpf = ctx.enter_context(tc.tile_pool(name="tpf", bufs=cfg["tpf_bufs"]))
            tp2 = ctx.enter_context(tc.tile_pool(name="tp2", bufs=13))
            pa = ctx.enter_context(tc.tile_pool(name="pa", bufs=1, space="PSUM"))
            op = ctx.enter_context(tc.tile_pool(name="op", bufs=1))

            ident = idp.tile([128, 128], F16, name="ident_t")
            warm = idp.tile([128, 2], F16, name="warm")

            xt = [xp.tile([128, P + 2], F16, tag=f"x{b}", name=f"x{b}") for b in (0, 1)]
            xe = {}
            for d in (0, 1):  # 0=up, 1=dn
                for b in (0, 1):
                    xe[(d, b)] = xp.tile(
                        [128, 514], F16, tag=f"xe{d}{b}", name=f"xe{d}{b}"
                    )
            wt = [
                wp.tile([128, 512], F16, tag=f"w{t}", name=f"w{t}") for t in range(K2)
            ]

            # --- input DMA schedule (SP + ACT queues run in parallel) ----
            # SP: ident + wt4 + all four xb0 quarters (block0 full-plane
            #     taps need the whole row early), then remaining weights in
            #     feed order.  ACT: xb1-hi + dn-xe + mid-stream weights;
            #     up-xe and xb1-lo ride behind the D drain.
            nc.sync.dma_start(ident[:], ident_d[:])
            nc.sync.dma_start(wt[4][:], wt_d[4])
            nc.sync.dma_start(wt[3][:], wt_d[3])
            nc.sync.dma_start(xt[0][:, 0:1026], xb_d[0, :, 0:1026])
            nc.sync.dma_start(xt[0][:, 1026:2050], xb_d[0, :, 1026:2050])
            nc.sync.dma_start(xt[0][:, 2050:3074], xb_d[0, :, 2050:3074])
            nc.sync.dma_start(xt[0][:, 3074 : P + 2], xb_d[0, :, 3074 : P + 2])
            nc.scalar.dma_start(xt[1][:, 2048:3074], xb_d[1, :, 2048:3074])
            nc.scalar.dma_start(xt[1][:, 3074 : P + 2], xb_d[1, :, 3074 : P + 2])
            nc.scalar.activation(
                warm[:], ident[:, 0:2], mybir.ActivationFunctionType.Copy
            )
            nc.sync.dma_start(wt[5][:], wt_d[5])
            nc.scalar.dma_start(xe[(1, 1)][:], xe_d[1, 1])  # dn, blk1
            nc.scalar.dma_start(xe[(1, 0)][:], xe_d[1, 0])  # dn, blk0
            for t in (7, 6, 8):
                nc.scalar.dma_start(wt[t][:], wt_d[t])
            for t in (1, 0, 2):
                nc.sync.dma_start(wt[t][:], wt_d[t])

            # --- helpers ------------------------------------------------
            def w_ap(t, rr_lo, rr_hi):
                w3 = wt[t][:].rearrange("p (rr j) -> p rr j", rr=8)
                nrr = rr_hi - rr_lo
                return (
                    w3[:, rr_lo:rr_hi]
                    .unsqueeze(2)
                    .broadcast_to([128, nrr, 8, 64])
                )

            def grid3(ap2):
                return ap2.rearrange("p (rr g j) -> p rr g j", g=8, j=64)

            def mult_main(dst2, blk, t, olo, ohi, eng=None):
                """dst2 covers output flat [olo, ohi) (multiples of 512)."""
                di, dj = _tap_geom(t)
                s = 1 + olo + 512 * di + dj
                xap = grid3(xt[blk][:, s : s + (ohi - olo)])
                oap = grid3(dst2)
                wap = w_ap(t, olo // 512, ohi // 512)
                (eng or pick_engine(ohi - olo)).tensor_mul(oap, xap, wap)

            def mult_boundary(dst2, blk, t, eng=None):
                di, dj = _tap_geom(t)
                src = xe[(0 if di < 0 else 1, blk)]
                rr_b = 0 if di < 0 else 7
                xap = src[:, 1 + dj : 1 + dj + 512].rearrange(
                    "p (g j) -> p g j", g=8
                )
                oap = dst2.rearrange("p (g j) -> p g j", g=8)
                w3 = wt[t][:].rearrange("p (rr j) -> p rr j", rr=8)
                wap = w3[:, rr_b].unsqueeze(1).broadcast_to([128, 8, 64])
                (eng or pick_engine(512)).tensor_mul(oap, xap, wap)

            def mult_full(pool, t):
                """one tap's full [0:4096] plane for block0 into a fresh
                full-width tile (1-2 ops)."""
                di, _ = _tap_geom(t)
                tmp = pool.tile([128, P], F16, tag="tmp", name="tmp")
                eng = pick_engine(P)
                if di == 0:
                    mult_main(tmp[:, 0:P], 0, t, 0, P, eng)
                elif di == -1:
                    mult_boundary(tmp[:, 0:512], 0, t, eng)
                    mult_main(tmp[:, 512:P], 0, t, 512, P, eng)
                else:
                    mult_main(tmp[:, 0 : P - 512], 0, t, 0, P - 512, eng)
                    mult_boundary(tmp[:, P - 512 : P], 0, t, eng)
                return tmp

            def region_ops(pool, blk, t, alo, ahi):
                """one tap's [alo, ahi) half-plane into a fresh half tile."""
                tmp = pool.tile([128, HALF], F16, tag="tmp", name="tmp")
                region_ops_into(tmp, blk, t, alo, ahi)
                return tmp

            def region_ops_into(tmp, blk, t, alo, ahi):
                di, _ = _tap_geom(t)
                eng = pick_engine(HALF)
                if di == 0:
                    mult_main(tmp[:, 0:HALF], blk, t, alo, ahi, eng)
                elif di == -1:
                    if alo == 0:
                        mult_boundary(tmp[:, 0:512], blk, t, eng)
                        mult_main(tmp[:, 512:HALF], blk, t, 512, ahi, eng)
                    else:
                        mult_main(tmp[:, 0:HALF], blk, t, alo, ahi, eng)
                else:
                    if ahi == P:
                        mult_main(tmp[:, 0 : HALF - 512], blk, t, alo, P - 512, eng)
                        mult_boundary(tmp[:, HALF - 512 : HALF], blk, t, eng)
                    else:
                        mult_main(tmp[:, 0:HALF], blk, t, alo, ahi, eng)

            def pe_acc_qh(ps_pair, tmp_half, h, first, last):
                """chunks for quarter h only ([h*1024, (h+1)*1024))."""
                for c0 in (h * QUAR, h * QUAR + 512):
                    nc.tensor.matmul(
                        ps_pair[h][:, (c0 % QUAR) : (c0 % QUAR) + 512],
                        ident[:],
                        tmp_half[:, c0 : c0 + 512],
                        start=first,
                        stop=last,
                        skip_group_check=True,
                    )

            def pe_acc_q(ps_pair, tmp_half, first, last):
                """identity-matmul a [128, 2048] tmp into a (q0, q1) pair of
                [128, 1024] PSUM quarter tiles, 512 per matmul."""
                for c0 in range(0, HALF, 512):
                    pq = ps_pair[c0 // QUAR]
                    o0 = c0 % QUAR
                    nc.tensor.matmul(
                        pq[:, o0 : o0 + 512],
                        ident[:],
                        tmp_half[:, c0 : c0 + 512],
                        start=first,
                        stop=last,
                        skip_group_check=True,
                    )

            # PSUM: 8 banks as four [128, 1024] quarter tiles.  (a0, a1)
            # host region A then C; (d0, d1) host D's partial then B.
            ps_a = [pa.tile([128, QUAR], F32, tag=f"ps_a{i}", name=f"ps_a{i}") for i in (0, 1)]
            ps_d = [pa.tile([128, QUAR], F32, tag=f"ps_d{i}", name=f"ps_d{i}") for i in (0, 1)]

            ob_A = op.tile([128, HALF], F16, tag="ob_A", name="ob_A")
            ob_B = op.tile([128, HALF], F16, tag="ob_B", name="ob_B")
            ob_C = [op.tile([128, 512], F16, tag=f"ob_C{i}", name=f"ob_C{i}") for i in range(4)]
            ob_D = op.tile([128, HALF], F16, tag="ob_D", name="ob_D")

            # --- schedule -------------------------------------------------
            # V/G multiply stream is emitted in FEED order (D4 interleaved
            # with early block0 taps so PE's in-order conveyor never
            # starves); PE matmuls are emitted separately in CONSUME order;
            # ACT drains/stores in their own order.  Tile dependency
            # tracking synchronizes across engines regardless of program
            # interleaving.
            D_PE = list(cfg["d_pe"])
            D_ADD = [t for t in cfg["tap_order"] if t not in D_PE]
            tap_order = list(cfg["tap_order"])

            tmps_d = {}
            tmps_b0 = {}
            tmps_c = {}

            # ---- V/G feed stream ----------------------------------------
            D_READY = {4: 2400.0, 3: 4300.0, 5: 2600.0, 7: 3600.0}
            B0_READY = {4: 2900.0, 3: 3000.0, 5: 3100.0}

            drain_est = [cfg["drain_est"]]
            n_added = [0]

            def d_add_tap(t):
                tmp_d = region_ops(tp2, 1, t, HALF, P)
                if cfg["pin_adds"]:
                    eng_busy["v"] += QUAR * EV + EV_FIX
                    nc.vector.tensor_add(
                        ob_D[:, 0:QUAR], ob_D[:, 0:QUAR], tmp_d[:, 0:QUAR]
                    )
                    eng_busy["g"] += QUAR * EG
                    nc.gpsimd.tensor_add(
                        ob_D[:, QUAR:HALF], ob_D[:, QUAR:HALF], tmp_d[:, QUAR:HALF]
                    )
                else:
                    for i in (0, 1):
                        pick_engine(QUAR).tensor_add(
                            ob_D[:, i * QUAR : (i + 1) * QUAR],
                            ob_D[:, i * QUAR : (i + 1) * QUAR],
                            tmp_d[:, i * QUAR : (i + 1) * QUAR],
                        )

            feed_pos = [0]

            def maybe_d_add():
                if n_added[0] < len(D_ADD) and feed_pos[0] >= cfg["add_pos"]:
                    d_add_tap(D_ADD[n_added[0]])
                    n_added[0] += 1

            def emit_d(t, quarters=False):
                ready_hint[0] = D_READY.get(t, 0.0)
                if quarters:
                    tmp = tp2.tile([128, HALF], F16, tag="tmp", name="tmp_d0")
                    mult_main(tmp[:, 0:QUAR], 1, t, HALF, HALF + QUAR)
                    mult_main(tmp[:, QUAR:HALF], 1, t, HALF + QUAR, P)
                else:
                    tmp = region_ops(tp2, 1, t, HALF, P)
                ready_hint[0] = 0.0
                tmps_d[t] = tmp

            def emit_b0(t, quarters=False):
                ready_hint[0] = B0_READY.get(t, 0.0)
                if quarters:
                    tmp = tpf.tile([128, P], F16, tag="tmp", name="tmp_b00")
                    mult_main(tmp[:, 0:QUAR], 0, t, 0, QUAR)
                    mult_main(tmp[:, QUAR:HALF], 0, t, QUAR, HALF)
                    mult_main(tmp[:, HALF:P], 0, t, HALF, P)
                else:
                    tmp = mult_full(tpf, t)
                ready_hint[0] = 0.0
                tmps_b0[t] = tmp
                feed_pos[0] += 1
                maybe_d_add()

            # ---- merged emission walk -----------------------------------
            # Feed (V/G): D4 interleaved with early block0 taps, then the
            # rest of block0 with C taps and D-adds interspersed, ending
            # with C's final (di=0) tap in eighths.
            # PE: warm, D4-partial, then REGION-MAJOR: all A chunks (A
            # closes and drains mid-kernel, freeing its banks for C), all B
            # chunks (from buffered block0 tmps, on D's banks), all C
            # chunks.  Only C's eighth-grained drain+store chain sits in
            # the tail.
            emit_d(D_PE[0], quarters=True)
            for _ in range(cfg["warm"]):
                nc.tensor.matmul(
                    ps_a[0][:, 0:128], ident[:], ident[:], start=True,
                    stop=True, skip_group_check=True,
                )
            pe_acc_q(ps_d, tmps_d[D_PE[0]], True, len(D_PE) == 1)
            emit_b0(tap_order[0], quarters=True)
            for k, t in enumerate(D_PE[1:]):
                emit_d(t)
                pe_acc_q(ps_d, tmps_d[t], False, k == len(D_PE) - 2)
                if 1 + k < K2:
                    emit_b0(tap_order[1 + k])
            # D partial closed: drain + deferred ACT loads
            for i in (0, 1):
                nc.scalar.activation(
                    ob_D[:, i * QUAR : (i + 1) * QUAR],
                    ps_d[i][:],
                    mybir.ActivationFunctionType.Copy,
                )
            nc.scalar.dma_start(xe[(0, 0)][:], xe_d[0, 0])  # up, blk0
            nc.scalar.dma_start(xt[1][:, 1536:2048], xb_d[1, :, 1536:2048])
            nc.scalar.dma_start(xe[(0, 1)][:], xe_d[0, 1])  # up, blk1
            nc.scalar.dma_start(xt[1][:, 0:1536], xb_d[1, :, 0:1536])

            # remaining block0 feed with PE A-chunks lagging one tap;
            # C taps interleave into the feed once xb1-lo has landed
            c_order = list(cfg["c_order"])
            ps_c = [
                pa.tile([128, QUAR], F32, tag=f"ps_a{i}", name=f"ps_c{i}")
                for i in (0, 1)
            ]
            n_c = [0]

            def emit_c_tap():
                t = c_order[n_c[0]]
                tmps_c[t] = region_ops(tp2, 1, t, 0, HALF)
                n_c[0] += 1

            done_b0 = len(D_PE)
            pe_a = [0]

            def pe_a_step():
                k = pe_a[0]
                t = tap_order[k]
                pe_acc_q(ps_a, tmps_b0[t][:, 0:HALF], k == 0, k == K2 - 1)
                pe_a[0] += 1
                kb = pe_a[0] - 1 - cfg["b_lag"]
                if kb >= 0:
                    pe_b_step(kb)

            pe_b = [0]

            def pe_b_step(k):
                t = tap_order[k]
                pe_acc_q(ps_d, tmps_b0[t][:, HALF:P], k == 0, k == K2 - 1)
                pe_b[0] = k + 1

            for fi, t in enumerate(tap_order[len(D_PE) :]):
                emit_b0(t)
                done_b0 += 1
                while pe_a[0] < done_b0 - 1:
                    pe_a_step()
                if done_b0 >= cfg["c_start"] and n_c[0] < cfg["c_early"]:
                    emit_c_tap()
            while n_added[0] < len(D_ADD):
                d_add_tap(D_ADD[n_added[0]])
                n_added[0] += 1
            while pe_a[0] < K2:
                pe_a_step()
            # A drains + store; C's PE pass takes over A's banks
            nc.scalar.activation(
                ob_A[:, 0:QUAR], ps_a[0][:], mybir.ActivationFunctionType.Copy
            )
            nc.scalar.activation(
                ob_A[:, QUAR:HALF], ps_a[1][:], mybir.ActivationFunctionType.Copy
            )
            nc.sync.dma_start(out[0, :, 0:HALF], ob_A[:])

            # C feed; then B PE pass from buffered tmps; then C PE pass
            while n_c[0] < K2 - 1:
                emit_c_tap()
            while pe_b[0] < K2:
                pe_b_step(pe_b[0])
            nc.scalar.activation(
                ob_B[:, 0:QUAR], ps_d[0][:], mybir.ActivationFunctionType.Copy
            )
            nc.scalar.activation(
                ob_B[:, QUAR:HALF], ps_d[1][:], mybir.ActivationFunctionType.Copy
            )
            nc.sync.dma_start(out[0, :, HALF:P], ob_B[:])
            for pos in range(K2 - 1):
                pe_acc_qh(ps_c, tmps_c[c_order[pos]], 0, pos == 0, False)
                if pos > 0:
                    pe_acc_qh(
                        ps_c, tmps_c[c_order[pos - 1]], 1, pos - 1 == 0, False
                    )
            pe_acc_qh(ps_c, tmps_c[c_order[K2 - 2]], 1, False, False)
            # final C tap in eighths: mult -> stop-chunk -> drain -> store
            t_last = c_order[-1]
            assert _tap_geom(t_last)[0] == 0, "last C tap must be di=0"
            tmp_cl = tp2.tile([128, HALF], F16, tag="tmp", name="tmp_cl")
            for e in range(4):
                lo = e * 512
                mult_main(tmp_cl[:, lo : lo + 512], 1, t_last, lo, lo + 512)
                nc.tensor.matmul(
                    ps_c[e // 2][:, (lo % QUAR) : (lo % QUAR) + 512],
                    ident[:],
                    tmp_cl[:, lo : lo + 512],
                    start=False,
                    stop=True,
                    skip_group_check=True,
                )
                nc.scalar.activation(
                    ob_C[e][:],
                    ps_c[e // 2][:, (lo % QUAR) : (lo % QUAR) + 512],
                    mybir.ActivationFunctionType.Copy,
                )
                nc.sync.dma_start(out[1, :, lo : lo + 512], ob_C[e][:])

            # ---- D store ------------------------------------------------
            nc.scalar.dma_start(out[1, :, HALF:P], ob_D[:])

    if legalize:
        _legalize_sync_waits(nc)
    return nc


_NC_CACHE = {}


def get_nc(legalize: bool = True, cfg: dict | None = None):
    key = ("nc_legal" if legalize else "nc_raw", repr(sorted((cfg or {}).items())))
    if key not in _NC_CACHE:
        _NC_CACHE[key] = _build(legalize, cfg)
    return _NC_CACHE[key]


# ---- host-side layout helpers ------------------------------------------


def _make_xb(xb: np.ndarray) -> np.ndarray:
    """[2, 128, P+2] fp16: xb[blk, wc*8+rb, 1 + rr*512 + g*64 + j] =
    x[(blk*8+g)*16 + wc, (rb*8+rr)*64 + j]; zero pad columns 0 and P+1."""
    xf = xb.reshape(2, 8, 16, 8, 8, 64)  # (blk, g, wc, rb, rr, j)
    perm = xf.transpose(0, 2, 3, 4, 1, 5).reshape(2, 128, P)
    outp = np.zeros((2, 128, P + 2), dtype=np.float16)
    outp[:, :, 1 : P + 1] = perm
    return outp


def _make_xe(xb: np.ndarray) -> np.ndarray:
    """[2(dir), 2(blk), 128, 514] fp16 boundary-row tiles:
    dir 0 (up): row rb*8 - 1 (zeros for rb == 0)
    dir 1 (dn): row rb*8 + 8 (zeros for rb == 7)"""
    x4 = xb.reshape(2, 8, 16, 64, 64)  # (blk, g, wc, r, j)
    res = np.zeros((2, 2, 16, 8, 8, 64), dtype=np.float16)  # (dir,blk,wc,rb,g,j)
    for rb in range(8):
        if rb > 0:
            res[0, :, :, rb] = x4[:, :, :, rb * 8 - 1, :].transpose(0, 2, 1, 3)
        if rb < 7:
            res[1, :, :, rb] = x4[:, :, :, rb * 8 + 8, :].transpose(0, 2, 1, 3)
    out = np.zeros((2, 2, 128, 514), dtype=np.float16)
    out[:, :, :, 1:513] = res.reshape(2, 2, 128, 512)
    return out


def _make_wt(wb: np.ndarray) -> np.ndarray:
    """[K2, 128, 512] fp16: wt[tap, wc*8+rb, rr*64+j] = w[wc, tap,
    (rb*8+rr)*64+j], with column-edge zeroing folded in."""
    w = wb.astype(np.float16).reshape(WC, K2, OH, OW).copy()
    for idx in range(K2):
        dj = idx % 3 - 1
        if dj == -1:
            w[:, idx, :, 0] = 0
        elif dj == 1:
            w[:, idx, :, OW - 1] = 0
    w = w.reshape(WC, K2, 8, 8, 64).transpose(1, 0, 2, 3, 4)
    return np.ascontiguousarray(w.reshape(K2, 128, 512))


def _unperm_out(op_: np.ndarray) -> np.ndarray:
    """inverse of the xb permutation: [2, 128, P] -> [C, P] f32."""
    o6 = op_.reshape(2, 16, 8, 8, 8, 64)  # (blk, wc, rb, rr, g, j)
    return o6.transpose(0, 4, 1, 2, 3, 5).reshape(C, P).astype(np.float32)


def kernel(x: np.ndarray, weight: np.ndarray) -> np.ndarray:
    x = np.ascontiguousarray(np.asarray(x, dtype=np.float32))
    weight = np.ascontiguousarray(np.asarray(weight, dtype=np.float32))
    assert x.shape == (B, C, H, W), x.shape
    assert weight.shape == (B, WC, K2, P), weight.shape

    nc = get_nc()
    in_maps = []
    for i in range(N_CORES):
        xi = x[i].reshape(C, P).astype(np.float16)
        in_maps.append(
            {
                "xb": _make_xb(xi),
                "xe": _make_xe(xi),
                "wt": _make_wt(weight[i]),
            }
        )
    try:
        res = run_bass_kernel_spmd(nc, in_maps, list(range(N_CORES)))
    except Exception:
        # the axon terminal occasionally reports a transient
        # NRT_EXEC_UNIT_UNRECOVERABLE for a known-good NEFF; retry once
        res = run_bass_kernel_spmd(nc, in_maps, list(range(N_CORES)))
    out = np.stack(
        [_unperm_out(res.results[i]["out"]) for i in range(N_CORES)], axis=0
    )
    return out.reshape(B, C, OH, OW)
